# revision 14
# baseline (speedup 1.0000x reference)
"""Multi-Query Attention kernel for 8x TRN2 NeuronCores (Bass/Tile).

Problem: x[B=2, L=2048, D=2048], Wq[2048,2048], Wk/Wv[128,2048] (MQA: one
shared K/V head), 16 query heads of dim 128.

Sharding: core c in [0,8): batch b = c//4, head-group g = c%4 (4 heads,
i.e. q-channels [512g, 512g+512)). K/V replicated per core (cheap).

Device-side layout strategy (everything "transposed" so that every matmul
contraction dim lands on SBUF partitions, with zero on-device transposes of
the big tensors):
  - host passes xT = x[b].T            [D, L]  (contraction dim D on rows)
  - host passes wqT/wkT/wvT = W.T      [D, out]
  - projections compute qT/kT/vT = W @ x.T = (x@W.T).T  -> [out_ch, L]
  - scores^T tile = (kT slice).T @ qT  -> [Lk, Lq]  (contraction d=128)
  - exp on ACT engine straight out of PSUM (scale fused), no max-subtract
    (inputs are small: |scores*scale| < ~6, exp is safe in fp32)
  - out^T = (V block).T @ attn^T accumulated over Lk blocks (V natural
    [L, d] obtained via 16 cheap 128x128 PE transposes of vT)
  - softmax denominator r = ones^T @ (sum_lk attn^T) via one tiny matmul
    per (head, lq), with the per-block partial sums accumulated on DVE
  - host transposes outT [512, L] back and concatenates core outputs

Matmuls run as float32r (full fp32 storage, reduced-precision multiply,
1 cycle/row at N>=256 vs 4 cycles/row for strict fp32).
"""

import os
from contextlib import ExitStack

import numpy as np

import concourse.bass as bass
import concourse.tile as tile
from concourse import bacc, masks, mybir
from concourse.bass_utils import run_bass_kernel_spmd

F32 = mybir.dt.float32
AF = mybir.ActivationFunctionType

B = 2
L = 2048
D = 2048  # d_model (contraction dim of projections)
HD = 128  # head dim
NH = 4  # heads per core
QC = NH * HD  # q-channels per core = 512
DC = D // 128  # d-model chunks of 128 = 16
NLT = 4  # l tiles of 512 (projection phase)
LKT = L // 128  # lk blocks of 128 = 16
NLQ = 4  # lq blocks of 512 (attention phase)
N_CORES = 8
SCALE = 1.0 / float(np.sqrt(HD))

# float32r: reduced-precision (tf32-like) matmul at full PE rate. Walrus
# requires every producer of an f32r-matmul operand to emit f32r, so all
# matmul-operand tiles are declared float32r and DRAM-side DMA APs are
# bitcast. Set BASS_MM_F32=1 to fall back to exact fp32 (4x slower on PE).
MM_DT = F32 if os.environ.get("BASS_MM_F32") else mybir.dt.float32r


def _mm(ap):
    return ap  # tiles already carry MM_DT


def _dr(ap):
    # bitcast a DRAM-side fp32 AP for DMA into an MM_DT tile
    return ap.bitcast(MM_DT) if MM_DT != F32 else ap


def build_kernel(ctx: ExitStack, tc: tile.TileContext, xT, wqT, wkT, wvT, bq, bk, bv, outT):
    nc = tc.nc

    persist = ctx.enter_context(tc.tile_pool(name="persist", bufs=1))
    qT = [persist.tile([128, L], MM_DT, tag=f"qT{h}", name=f"qT{h}") for h in range(NH)]  # [d, l]
    kT = persist.tile([128, L], MM_DT, tag="kT", name="kT")  # [d, l]
    vN = persist.tile([128, L], MM_DT, tag="vN", name="vN")  # block j: [:, 128j:+128] = V[128j:+128, :]
    ones = persist.tile([128, 128], F32, tag="ones", name="ones")
    ident = persist.tile([128, 128], F32, tag="ident", name="ident")
    bq_sb = persist.tile([128, NH], F32, tag="bq", name="bq")
    bk_sb = persist.tile([128, 1], F32, tag="bk", name="bk")
    bv_sb = persist.tile([128, 1], F32, tag="bv", name="bv")

    nc.vector.memset(ones[:], 1.0)
    masks.make_identity(nc, ident[:])
    nc.sync.dma_start(out=bq_sb[:], in_=bq)
    nc.sync.dma_start(out=bk_sb[:], in_=bk)
    nc.sync.dma_start(out=bv_sb[:], in_=bv)

    # ---------------- Phase B: projections qT/kT/vT = W @ x^T ----------------
    with (
        tc.tile_pool(name="wq", bufs=1) as wqp,
        tc.tile_pool(name="wkv", bufs=1) as wkvp,
        tc.tile_pool(name="xt", bufs=2) as xtp,
        tc.tile_pool(name="pj", bufs=1, space="PSUM") as pjp,
        tc.tile_pool(name="vt", bufs=1) as vtp,
    ):
        # one tile per d-chunk so each matmul waits on a single DMA sem
        wq_ch = [wqp.tile([128, QC], MM_DT, tag=f"wqc{k}", name=f"wqc{k}") for k in range(DC)]
        wk_ch = [wkvp.tile([128, HD], MM_DT, tag=f"wkc{k}", name=f"wkc{k}") for k in range(DC)]
        wv_ch = [wkvp.tile([128, HD], MM_DT, tag=f"wvc{k}", name=f"wvc{k}") for k in range(DC)]
        vT = vtp.tile([128, L], F32, tag="vT", name="vT")
        for k in range(DC):
            nc.sync.dma_start(out=wq_ch[k][:], in_=_dr(wqT[k * 128:(k + 1) * 128, :]))
            nc.sync.dma_start(out=wk_ch[k][:], in_=_dr(wkT[k * 128:(k + 1) * 128, :]))
            nc.sync.dma_start(out=wv_ch[k][:], in_=_dr(wvT[k * 128:(k + 1) * 128, :]))

        for lt in range(NLT):
            ls = slice(lt * 512, (lt + 1) * 512)
            # 6 concurrent PSUM accumulation groups: Q0..Q3, K, V
            psq = [pjp.tile([128, 512], F32, tag=f"pjq{t}", name=f"pjq{t}") for t in range(NH)]
            psk = pjp.tile([128, 512], F32, tag="pjk", name="pjk")
            psv = pjp.tile([128, 512], F32, tag="pjv", name="pjv")
            for k in range(DC):
                xc = xtp.tile([128, 512], MM_DT, tag=f"xt{k % 4}", name=f"xt{k % 4}")
                nc.sync.dma_start(out=xc[:], in_=_dr(xT[k * 128:(k + 1) * 128, ls]))
                st = k == 0
                sp = k == DC - 1
                for t in range(NH):
                    nc.tensor.matmul(
                        psq[t][:],
                        lhsT=_mm(wq_ch[k][:, t * 128:(t + 1) * 128]),
                        rhs=_mm(xc[:]),
                        start=st,
                        stop=sp,
                    )
                nc.tensor.matmul(psk[:], lhsT=_mm(wk_ch[k][:]), rhs=_mm(xc[:]), start=st, stop=sp)
                nc.tensor.matmul(psv[:], lhsT=_mm(wv_ch[k][:]), rhs=_mm(xc[:]), start=st, stop=sp)
            for t in range(NH):
                nc.scalar.activation(qT[t][:, ls], psq[t][:], AF.Identity, bias=bq_sb[:, t:t + 1])
            nc.scalar.activation(kT[:, ls], psk[:], AF.Identity, bias=bk_sb[:, 0:1])
            nc.scalar.activation(vT[:, ls], psv[:], AF.Identity, bias=bv_sb[:, 0:1])

        # -------- Phase C: V natural [L, d] via 16 PE transposes of vT --------
        with tc.tile_pool(name="tp", bufs=2, space="PSUM") as tpp:
            for j in range(LKT):
                pt = tpp.tile([128, 128], F32, tag="tp", name="tp")
                nc.tensor.transpose(pt[:], vT[:, j * 128:(j + 1) * 128], ident[:])
                nc.scalar.activation(vN[:, j * 128:(j + 1) * 128], pt[:], AF.Identity)

    # ---------------- Phase D: attention ----------------
    with (
        tc.tile_pool(name="sps", bufs=2, space="PSUM") as sps,  # 2 x [128,1024] = 4 banks
        tc.tile_pool(name="avp", bufs=4, space="PSUM") as avp,  # 4 x [128,512] = 4 banks
        tc.tile_pool(name="att", bufs=8) as attp,
        tc.tile_pool(name="rr", bufs=2) as rrp,
        tc.tile_pool(name="fin", bufs=4) as finp,
    ):
        POOL_LK = {2, 5, 7, 10, 13, 15}  # r-partials accumulated on GpSimd
        for lq in range(NLQ):
            qs = slice(lq * 512, (lq + 1) * 512)
            psA = [avp.tile([128, 512], F32, tag="av", name="av") for _ in range(NH)]
            rres_d = rrp.tile([128, NH * 512], F32, tag="rrd", name="rrd")
            rres_p = rrp.tile([128, NH * 512], F32, tag="rrp", name="rrp")
            first_d = True
            first_p = True
            for lk in range(LKT):
                ks = slice(lk * 128, (lk + 1) * 128)
                on_pool = lk in POOL_LK
                for p in range(2):  # head pairs -> [128, 1024] score tiles
                    ss = sps.tile([128, 1024], F32, tag="sps", name="sps")
                    for j in range(2):
                        h = 2 * p + j
                        nc.tensor.matmul(
                            ss[:, j * 512:(j + 1) * 512],
                            lhsT=_mm(kT[:, ks]),
                            rhs=_mm(qT[h][:, qs]),
                            start=True,
                            stop=True,
                        )
                    at = attp.tile([128, 1024], MM_DT, tag="att", name="att")
                    nc.scalar.activation(at[:], ss[:], AF.Exp, scale=SCALE)
                    for j in range(2):
                        h = 2 * p + j
                        nc.tensor.matmul(
                            psA[h][:],
                            lhsT=_mm(vN[:, ks]),
                            rhs=_mm(at[:, j * 512:(j + 1) * 512]),
                            start=(lk == 0),
                            stop=(lk == LKT - 1),
                        )
                    pr_slice = slice(p * 1024, (p + 1) * 1024)
                    at_f32 = at[:].bitcast(F32)
                    if on_pool:
                        if first_p and p == 1:
                            first_p = False
                        if lk in POOL_LK and (lk == min(POOL_LK)):
                            nc.gpsimd.tensor_copy(rres_p[:, pr_slice], at_f32)
                        else:
                            nc.gpsimd.tensor_add(rres_p[:, pr_slice], rres_p[:, pr_slice], at_f32)
                    else:
                        if lk == 0:
                            nc.vector.tensor_copy(rres_d[:, pr_slice], at_f32)
                        else:
                            nc.vector.tensor_add(rres_d[:, pr_slice], rres_d[:, pr_slice], at_f32)
            # combine the two accumulators, then r per head via fp32 ones-matmul
            nc.vector.tensor_add(rres_d[:], rres_d[:], rres_p[:])
            for h in range(NH):
                # r replicated to all 128 partitions: (ones 128x128)^T @ rres
                pr = sps.tile([128, 512], F32, tag="sps", name="sps")
                nc.tensor.matmul(
                    pr[:],
                    lhsT=ones[:],
                    rhs=rres_d[:, h * 512:(h + 1) * 512],
                    start=True,
                    stop=True,
                )
                rinv = finp.tile([128, 512], F32, tag="rinv", name="rinv")
                nc.vector.reciprocal(rinv[:], pr[:])
                ot = finp.tile([128, 512], F32, tag="ot", name="ot")
                nc.vector.tensor_mul(ot[:], psA[h][:], rinv[:])
                nc.sync.dma_start(out=outT[h * 128:(h + 1) * 128, qs], in_=ot[:])


_NC_CACHE = None


def build_nc():
    global _NC_CACHE
    if _NC_CACHE is not None:
        return _NC_CACHE
    nc = bacc.Bacc("TRN2", target_bir_lowering=False, debug=False)
    xT = nc.dram_tensor("xT", [D, L], F32, kind="ExternalInput").ap()
    wqT = nc.dram_tensor("wqT", [D, QC], F32, kind="ExternalInput").ap()
    wkT = nc.dram_tensor("wkT", [D, HD], F32, kind="ExternalInput").ap()
    wvT = nc.dram_tensor("wvT", [D, HD], F32, kind="ExternalInput").ap()
    bq = nc.dram_tensor("bq", [128, NH], F32, kind="ExternalInput").ap()
    bk = nc.dram_tensor("bk", [128, 1], F32, kind="ExternalInput").ap()
    bv = nc.dram_tensor("bv", [128, 1], F32, kind="ExternalInput").ap()
    outT = nc.dram_tensor("outT", [QC, L], F32, kind="ExternalOutput").ap()
    with tile.TileContext(nc) as tc, ExitStack() as ctx:
        build_kernel(ctx, tc, xT, wqT, wkT, wvT, bq, bk, bv, outT)
    nc.compile()
    _NC_CACHE = nc
    return nc


def make_in_maps(x, Wq_w, Wq_b, Wk_w, Wk_b, Wv_w, Wv_b):
    """Host-side sharding/relayout. Returns one input map per core."""
    x = np.asarray(x, dtype=np.float32)
    Wq_w = np.asarray(Wq_w, dtype=np.float32)
    Wq_b = np.asarray(Wq_b, dtype=np.float32)
    Wk_w = np.asarray(Wk_w, dtype=np.float32)
    Wk_b = np.asarray(Wk_b, dtype=np.float32)
    Wv_w = np.asarray(Wv_w, dtype=np.float32)
    Wv_b = np.asarray(Wv_b, dtype=np.float32)

    xTs = [np.ascontiguousarray(x[b].T) for b in range(B)]
    wkT = np.ascontiguousarray(Wk_w.T)
    wvT = np.ascontiguousarray(Wv_w.T)
    bk = np.ascontiguousarray(Wk_b.reshape(128, 1))
    bv = np.ascontiguousarray(Wv_b.reshape(128, 1))
    in_maps = []
    for c in range(N_CORES):
        b, g = divmod(c, B * 2)  # b = c // 4, g = c % 4
        wqT_g = np.ascontiguousarray(Wq_w[g * QC:(g + 1) * QC, :].T)
        bq_g = np.ascontiguousarray(Wq_b[g * QC:(g + 1) * QC].reshape(NH, 128).T)
        in_maps.append(
            {
                "xT": xTs[b],
                "wqT": wqT_g,
                "wkT": wkT,
                "wvT": wvT,
                "bq": bq_g,
                "bk": bk,
                "bv": bv,
            }
        )
    return in_maps


def assemble_output(results):
    out = np.empty((B, L, D), dtype=np.float32)
    for c in range(N_CORES):
        b, g = divmod(c, B * 2)
        out[b, :, g * QC:(g + 1) * QC] = results[c]["outT"].T
    return out


def kernel(**inputs) -> np.ndarray:
    nc = build_nc()
    in_maps = make_in_maps(**inputs)
    res = run_bass_kernel_spmd(nc, in_maps, core_ids=list(range(N_CORES)))
    return assemble_output(res.results)
